# revision 40
# baseline (speedup 1.0000x reference)
"""Trainium2 Bass kernel for the Boat Dynamic System — rank-8 CP, fp16, v7.

Math: out[b, c] = s~^T Q_c s~ with s~ = (1, u, v, r, Pf); pro/rud folded on
host. The 4x5x5 tensor {Q_c} is CP-decomposed (Gauss-Newton, exact fit) as
Q_c = sum_{j=0..7} lam[c,j] w_j w_j^T, then W is rounded to fp16 and
(lam, bias) are REFIT in f64 so quantization cancels to first order.

v11 balances the elementwise work across ACT and DVE (the only two engines
with a PSUM read port; DVE has ONE such port so tensor ops may read at most
one PSUM operand), moves every DMA to HWDGE (sync/scalar queues), and warms
the PE's HAM clock gate with dummy matmuls on memset data during the DMA
head so the real matmul stream runs at 2.4 GHz instead of 1.2:
  - ACT: Square(y + b) for 13 of the 16 square-chunks (its only func ->
    the activation table never reloads, 1.3us each)
  - DVE: for chunks in DVE_ZB, the B-half square runs two-stage:
    t = fp16(yB + bB) via tensor_scalar (PSUM->SBUF), then zB = t*t via
    tensor_tensor (all-SBUF fp16 -> 2x DVE mode)
  - DVE: all M2-output PSUM->SBUF fp16 casts (tensor_copy)

Per [128, 1024] chunk (8 per core): 2 fp16 matmuls wA -> yA psum, 2 wB ->
yB, ACT/DVE squares -> fp16 SBUF, then (pipelined one chunk behind) 4
accumulating lam matmuls back into yA's banks, DVE cast, sync HWDGE DMA
out per chunk (the last chunk drains at 512 granularity). Host does all
layout permutes + f32 upcast. Weights+biases ride one packed [128,516]
fp16 DMA on the ACT ring (bias columns bitcast to f32 on device).

A raw-bass (no Tile) variant is kept behind BOAT_RAW=1; the Tile scheduler
measured faster (its dynamic ordering absorbs the ACT/DVE latency chain
better than the 8-PSUM-bank static schedule allows).
"""

import os

import numpy as np

NCORES = 8
B = 2097152
BS = B // NCORES          # 262144 rows per core
DT = 0.01
NTILES = 4                # [128, 2048] tiles per core
TILE_F = 2048             # free dim per tile (512 batch x 4 comps)
NCHUNK = 512              # matmul free size (one PSUM f32 bank)
CPT = TILE_F // NCHUNK    # 4 chunks per tile
NFUNC = 8

_NC_CACHE = {}
LAST_RESULT = [None]

# ---------------------------------------------------------------- host math

_MONO_QUAD = [(0, 0), (0, 1), (0, 2), (0, 3), (1, 1), (1, 2), (1, 3),
              (2, 2), (2, 3), (3, 3)]


def _build_Q(t, cmd, coeffs):
    idx = int(np.round(float(np.asarray(t).reshape(-1)[0]) / DT))
    pro = float(cmd[idx, 0])
    rud = float(cmd[idx, 1])
    cf = np.asarray(coeffs, dtype=np.float64)
    ceff = cf[:, 0:15] + pro * cf[:, 15:30] + rud * cf[:, 30:45]  # [4,15]

    Q = np.zeros((4, 5, 5))
    Q[:, 0, 0] = ceff[:, 0]
    for f in range(4):
        Q[:, 0, 1 + f] += ceff[:, 1 + f] / 2
        Q[:, 1 + f, 0] += ceff[:, 1 + f] / 2
    for k, (x, y) in enumerate(_MONO_QUAD):
        m = 5 + k
        if x == y:
            Q[:, 1 + x, 1 + x] += ceff[:, m]
        else:
            Q[:, 1 + x, 1 + y] += ceff[:, m] / 2
            Q[:, 1 + y, 1 + x] += ceff[:, m] / 2
    return Q


def _cp_decompose(Q, N=NFUNC, restarts=40, iters=200, seed=0, tol=1e-11):
    """Gauss-Newton (LM) exact symmetric CP fit: Q_c = sum_j lam_cj w_j w_j^T."""
    rng = np.random.default_rng(seed)
    qn = np.linalg.norm(Q)
    best = None
    iu = np.triu_indices(5)
    wts = np.where(iu[0] == iu[1], 1.0, np.sqrt(2.0))
    a_idx, b_idx = iu

    def resid(W, lam):
        outer = np.einsum('ja,jb->jab', W, W)
        R = Q - np.einsum('cj,jab->cab', lam, outer)
        return (R[:, a_idx, b_idx] * wts).ravel()

    NJ = N * 5 + 4 * N
    for trial in range(restarts):
        W = rng.standard_normal((N, 5))
        lam = rng.standard_normal((4, N)) * 0.1
        mu = 1e-6
        for _ in range(iters):
            r = resid(W, lam)
            res = np.linalg.norm(r)
            J = np.zeros((60, NJ))
            for j in range(N):
                for e in range(5):
                    contrib = (np.where(a_idx == e, W[j, b_idx], 0.0)
                               + np.where(b_idx == e, W[j, a_idx], 0.0)) * wts
                    J[:, j * 5 + e] = (-lam[:, j][:, None]
                                       * contrib[None, :]).ravel()
            outerj = W[:, a_idx] * W[:, b_idx] * wts
            for c in range(4):
                for j in range(N):
                    col = np.zeros((4, 15))
                    col[c] = -outerj[j]
                    J[:, N * 5 + c * N + j] = col.ravel()
            JTJ = J.T @ J
            g = J.T @ r
            ok = False
            for _ in range(40):
                try:
                    step = np.linalg.solve(JTJ + mu * np.eye(NJ), -g)
                except np.linalg.LinAlgError:
                    mu *= 10
                    continue
                Wn = W + step[:N * 5].reshape(N, 5)
                lamn = lam + step[N * 5:].reshape(4, N)
                if np.linalg.norm(resid(Wn, lamn)) < res:
                    W, lam = Wn, lamn
                    mu = max(mu / 3, 1e-12)
                    ok = True
                    break
                mu *= 10
                if mu > 1e12:
                    break
            if not ok:
                break
            if res < tol * qn * 0.1:
                break
        res = np.linalg.norm(resid(W, lam)) / qn
        if res < tol:
            amp = float(np.abs(lam).sum(axis=1).max() * (np.abs(W).max() ** 2))
            if best is None or amp < best[3]:
                best = (W.copy(), lam.copy(), res, amp)
            if trial >= 2 and best[3] < 60:
                break
    if best is None:
        raise RuntimeError("CP decomposition failed to converge")
    return best


def _refit_fp16(Q, W, lam):
    """Round W[:,1:5] to fp16; refit (lam, bias) in f64; quantize lam to
    fp16 with a final bias refit."""
    N = W.shape[0]
    iu = np.triu_indices(5)
    wts = np.where(iu[0] == iu[1], 1.0, np.sqrt(2.0))
    Tq = Q[:, iu[0], iu[1]] * wts                     # [4,15]

    Wf = W.astype(np.float64).copy()
    Wf[:, 1:5] = Wf[:, 1:5].astype(np.float16)
    b = Wf[:, 0].copy()

    def gram(Wfull):
        return np.einsum('ja,jb->jab', Wfull, Wfull)[:, iu[0], iu[1]] * wts

    def fit_lam(Wfull):
        G = gram(Wfull)
        lam2, *_ = np.linalg.lstsq(G.T, Tq.T, rcond=None)
        return lam2.T, G

    def refit_bias(lamx, b):
        for _ in range(100):
            Wf[:, 0] = b
            G = gram(Wf)
            R = Tq - lamx @ G
            J = np.zeros((60, N))
            for j in range(N):
                dG = np.zeros((5, 5))
                dG[0, :] += Wf[j]
                dG[:, 0] += Wf[j]
                J[:, j] = np.outer(lamx[:, j], dG[iu[0], iu[1]] * wts).ravel()
            step, *_ = np.linalg.lstsq(J, R.ravel(), rcond=None)
            b = b + step
            if np.linalg.norm(step) < 1e-13:
                break
        return b

    for _ in range(3):
        Wf[:, 0] = b
        lam2, _ = fit_lam(Wf)
        b = refit_bias(lam2, b)
    lam16 = lam2.astype(np.float16).astype(np.float64)
    b = refit_bias(lam16, b)
    Wf[:, 0] = b
    return Wf, b, lam16


def _host_weights(t, cmd, coeffs):
    """Device weights: 4x [128,128] block-diag kron mats (fp16) + biases.

    biasp columns: 0 = bA (A-half Square bias), 1 = bB (B-half bias).
    """
    Q = _build_Q(t, cmd, coeffs)
    W, lam, res, amp = _cp_decompose(Q)
    Wf, bias, lam16 = _refit_fp16(Q, W, lam)

    I32 = np.eye(32)
    wA = np.kron(I32, Wf[0:4, 1:5].T).astype(np.float16)    # [128,128]
    wB = np.kron(I32, Wf[4:8, 1:5].T).astype(np.float16)
    lamA = np.kron(I32, lam16[:, 0:4].T).astype(np.float16)
    lamB = np.kron(I32, lam16[:, 4:8].T).astype(np.float16)
    biasp = np.stack([
        np.tile(bias[0:4], 32),
        np.tile(bias[4:8], 32),
    ], axis=1).astype(np.float32)                           # [128, 2]
    return wA, wB, lamA, lamB, biasp


# ---------------------------------------------------------------- device

def _build_nc_raw():
    """Raw-bass (no Tile) build: manual semaphores, ~170 instructions.

    Engine programs:
      SP:  3 in-DMAs, then per chunk wait(cast) -> out-DMA, final drain wait
      PE:  per chunk h: M1 (4 mm) -> inc sP1; M2 of h-1 (4 mm) -> inc sP2
      ACT: packed weight DMA, warm Square, per chunk zA (+zB for non-DVE
           chunks), inc sZA / sZB_ACT
      DVE: two-stage zB for DVE_ZB chunks (inc sZB_DVE), cast of chunk h-1
           after sP2 (inc sCast)
    """
    import concourse.bacc as bacc
    import concourse.mybir as mybir

    nc = bacc.Bacc("TRN2", target_bir_lowering=False, debug=False)
    f32 = mybir.dt.float32
    f16 = mybir.dt.float16
    Square = mybir.ActivationFunctionType.Square
    Alu = mybir.AluOpType

    xt_d = nc.dram_tensor("xt", [128, NTILES * TILE_F], f16,
                          kind="ExternalInput")
    # wpack cols 0:512 = wA|wB|lamA|lamB f16; cols 512:516 = biasp bitcast
    # ([128, 2] f32: bA, bB)
    wpack_d = nc.dram_tensor("wpack", [128, 516], f16, kind="ExternalInput")
    out = nc.dram_tensor("out", [128, NTILES * TILE_F], f16,
                         kind="ExternalOutput")

    NH = NTILES * TILE_F // 1024          # 8 chunks of 1024 cols
    DVE_ZB = (0, 7)                       # chunks whose B-half squares on DVE
    NWARM = 8                             # dummy MMs to lift the HAM clock gate

    # zB(h) completion bookkeeping: which engine + that engine's count
    zb_eng = ["dve" if h in DVE_ZB else "act" for h in range(NH)]
    zb_cnt = []
    na = nd = 0
    for h in range(NH):
        if zb_eng[h] == "act":
            na += 1
            zb_cnt.append(na)
        else:
            nd += 1
            zb_cnt.append(nd)

    NYA = 2                               # yA psum ring depth (4 banks)
    NYB = 2                               # yB psum ring depth (4 banks)

    from contextlib import ExitStack
    with ExitStack() as stack:
        ec = stack.enter_context
        sIN0 = ec(nc.semaphore("sIN0"))
        sIN1 = ec(nc.semaphore("sIN1"))
        sIN2 = ec(nc.semaphore("sIN2"))
        sW = ec(nc.semaphore("sW"))
        sOUT = ec(nc.semaphore("sOUT"))
        sP1a = ec(nc.semaphore("sP1a"))   # yA M1 pair done
        sP1b = ec(nc.semaphore("sP1b"))   # yB M1 pair done
        sP2 = ec(nc.semaphore("sP2"))
        sZA = ec(nc.semaphore("sZA"))
        sZBa = ec(nc.semaphore("sZBa"))
        sZBd = ec(nc.semaphore("sZBd"))
        sTB = ec(nc.semaphore("sTB"))
        sCast = ec(nc.semaphore("sCast"))
        xt = ec(nc.sbuf_tensor("xt_s", [128, NTILES * TILE_F], f16))
        wpack = ec(nc.sbuf_tensor("wpack_s", [128, 516], f16))
        onat = ec(nc.sbuf_tensor("onat_s", [128, NTILES * TILE_F], f16))
        zA_s = ec(nc.sbuf_tensor("zA_s", [128, 2 * 1024], f16))
        zB_s = ec(nc.sbuf_tensor("zB_s", [128, 2 * 1024], f16))
        tB_s = ec(nc.sbuf_tensor("tB_s", [128, 2 * 1024], f16))
        warm_s = ec(nc.sbuf_tensor("warm_s", [128, 1], f32))
        yA0 = ec(nc.psum_tensor("yA0", [128, 1024], f32))
        yA1 = ec(nc.psum_tensor("yA1", [128, 1024], f32))
        yB0 = ec(nc.psum_tensor("yB0", [128, 1024], f32))
        yB1 = ec(nc.psum_tensor("yB1", [128, 1024], f32))
        wA = wpack[:, 0:128]
        wB = wpack[:, 128:256]
        lamA = wpack[:, 256:384]
        lamB = wpack[:, 384:512]
        biasp = wpack[:, 512:516].bitcast(f32)   # [128, 2] f32
        yAs = [yA0, yA1]
        yBs = [yB0, yB1]
        zAs = [zA_s[:, 0:1024], zA_s[:, 1024:2048]]
        zBs = [zB_s[:, 0:1024], zB_s[:, 1024:2048]]
        tBs = [tB_s[:, 0:1024], tB_s[:, 1024:2048]]
        # in-DMA completion semaphore per chunk (one sem per DMA: concurrent
        # DMA completions on one sem are unordered)
        in_need = [sIN0, sIN1, sIN1, sIN1, sIN2, sIN2, sIN2, sIN2]

        # ---- SP program: x in, then per-chunk out (weights ride the ACT
        # HWDGE ring in parallel)
        nc.sync.dma_start(out=xt[:, 0:1024],
                          in_=xt_d[:, 0:1024]).then_inc(sIN0, 16)
        nc.sync.dma_start(out=xt[:, 1024:4096],
                          in_=xt_d[:, 1024:4096]).then_inc(sIN1, 16)
        nc.sync.dma_start(out=xt[:, 4096:8192],
                          in_=xt_d[:, 4096:8192]).then_inc(sIN2, 16)
        for h in range(NH - 1):
            nc.sync.wait_ge(sCast, h + 1)
            nc.sync.dma_start(
                out=out[:, h * 1024:(h + 1) * 1024],
                in_=onat[:, h * 1024:(h + 1) * 1024],
            ).then_inc(sOUT, 16)
        # last chunk drains at 512 granularity (casts split likewise)
        for u in range(2):
            nc.sync.wait_ge(sCast, NH + u)
            s0 = (NH - 1) * 1024 + u * 512
            nc.sync.dma_start(
                out=out[:, s0:s0 + 512], in_=onat[:, s0:s0 + 512],
            ).then_inc(sOUT, 16)
        nc.sync.wait_ge(sOUT, 16 * (NH + 1))

        # ---- ACT program
        nc.scalar.dma_start(out=wpack[:], in_=wpack_d[:, :]).then_inc(sW, 16)
        nc.scalar.wait_ge(sW, 16)
        nc.scalar.activation(out=warm_s[:], in_=biasp[:, 0:1], func=Square,
                             bias=0.0, scale=1.0)
        for h in range(NH):
            nc.scalar.wait_ge(sP1a, h + 1)
            if h >= 2:
                nc.scalar.wait_ge(sP2, h - 1)   # zA buf WAR vs M2A(h-2)
            nc.scalar.activation(out=zAs[h % 2], in_=yAs[h % NYA][:],
                                 func=Square, bias=biasp[:, 0:1],
                                 scale=1.0).then_inc(sZA, 1)
            if zb_eng[h] == "act":
                nc.scalar.wait_ge(sP1b, h + 1)
                nc.scalar.activation(out=zBs[h % 2], in_=yBs[h % NYB][:],
                                     func=Square, bias=biasp[:, 1:2],
                                     scale=1.0).then_inc(sZBa, 1)

        # ---- DVE program (casts lead; DVE zB only on fill/tail chunks)
        for h in range(NH):
            if h >= 1:
                nc.vector.wait_ge(sP2, h)
                if h - 1 == NH - 1:
                    break
                nc.vector.tensor_copy(
                    out=onat[:, (h - 1) * 1024:h * 1024],
                    in_=yAs[(h - 1) % NYA][:],
                ).then_inc(sCast, 1)
            if zb_eng[h] == "dve":
                nc.vector.wait_ge(sP1b, h + 1)
                if h >= 2:
                    nc.vector.wait_ge(sP2, h - 1)   # zB buf WAR
                nc.vector.tensor_scalar(
                    out=tBs[h % 2], in0=yBs[h % NYB][:],
                    scalar1=biasp[:, 1:2], scalar2=None, op0=Alu.add,
                ).then_inc(sTB, 1)
                nc.vector.wait_ge(sTB, zb_cnt[h])   # same-engine RAW on tB
                nc.vector.tensor_tensor(
                    out=zBs[h % 2], in0=tBs[h % 2], in1=tBs[h % 2],
                    op=Alu.mult,
                ).then_inc(sZBd, 1)
        nc.vector.wait_ge(sP2, NH)
        for u in range(2):                  # split final cast for the drain
            s0 = (NH - 1) * 1024 + u * 512
            nc.vector.tensor_copy(
                out=onat[:, s0:s0 + 512],
                in_=yAs[(NH - 1) % NYA][:, u * 512:(u + 1) * 512],
            ).then_inc(sCast, 1)

        # ---- PE program
        # dummy matmuls at program start: keep the PE busy from ~7us so the
        # HAM clock gate reaches 8/8 before the real stream begins
        nc.tensor.wait_ge(sW, 16)
        for i in range(NWARM):
            nc.tensor.matmul(
                out=yB0[:, 0:512], lhsT=wA, rhs=wpack[:, 0:512],
                start=True, stop=True,
            )

        def m1(h):
            nc.tensor.wait_ge(in_need[h], 16)
            if h >= NYA:
                nc.tensor.wait_ge(sCast, h - NYA + 1)    # yA(h-NYA) free
            if h >= NYB:                                 # yB(h-NYB) free
                if zb_eng[h - NYB] == "act":
                    nc.tensor.wait_ge(sZBa, zb_cnt[h - NYB])
                else:
                    nc.tensor.wait_ge(sZBd, zb_cnt[h - NYB])
            c0 = h * 1024
            for u in range(2):
                mm = nc.tensor.matmul(
                    out=yAs[h % NYA][:, u * 512:(u + 1) * 512], lhsT=wA,
                    rhs=xt[:, c0 + u * 512:c0 + (u + 1) * 512],
                    start=True, stop=True,
                )
            mm.then_inc(sP1a, 1)
            for u in range(2):
                mm = nc.tensor.matmul(
                    out=yBs[h % NYB][:, u * 512:(u + 1) * 512], lhsT=wB,
                    rhs=xt[:, c0 + u * 512:c0 + (u + 1) * 512],
                    start=True, stop=True,
                )
            mm.then_inc(sP1b, 1)

        def m2(h):
            nc.tensor.wait_ge(sZA, h + 1)
            if zb_eng[h] == "act":
                nc.tensor.wait_ge(sZBa, zb_cnt[h])
            else:
                nc.tensor.wait_ge(sZBd, zb_cnt[h])
            for u in range(2):
                nc.tensor.matmul(
                    out=yAs[h % NYA][:, u * 512:(u + 1) * 512], lhsT=lamA,
                    rhs=zAs[h % 2][:, u * 512:(u + 1) * 512],
                    start=True, stop=False,
                )
            for u in range(2):
                mm = nc.tensor.matmul(
                    out=yAs[h % NYA][:, u * 512:(u + 1) * 512], lhsT=lamB,
                    rhs=zBs[h % 2][:, u * 512:(u + 1) * 512],
                    start=False, stop=True,
                )
            mm.then_inc(sP2, 1)

        for h in range(NH):
            m1(h)
            if h >= 1:
                m2(h - 1)
        m2(NH - 1)

    nc.finalize()
    return nc


def _build_nc():
    import concourse.bacc as bacc
    import concourse.mybir as mybir
    import concourse.tile as tile

    nc = bacc.Bacc("TRN2", target_bir_lowering=False, debug=False)
    f32 = mybir.dt.float32
    f16 = mybir.dt.float16
    Square = mybir.ActivationFunctionType.Square
    Alu = mybir.AluOpType

    # [q, (T g)]: per tile, 4KB contiguous per partition
    xt_d = nc.dram_tensor("xt", [128, NTILES * TILE_F], f16,
                          kind="ExternalInput")
    wpack_d = nc.dram_tensor("wpack", [128, 516], f16, kind="ExternalInput")
    out = nc.dram_tensor("out", [128, NTILES * TILE_F], f16,
                         kind="ExternalOutput")

    NH = NTILES * TILE_F // 1024          # 8 chunks of 1024 cols

    with tile.TileContext(nc) as tc:
        with (
            tc.tile_pool(name="consts", bufs=1) as cpool,
            tc.tile_pool(name="xt", bufs=4) as xtp,
            tc.tile_pool(name="z", bufs=2) as zp,
            tc.tile_pool(name="onat", bufs=4) as onp_,
            tc.tile_pool(name="pa", bufs=2, space="PSUM") as pap,
            tc.tile_pool(name="pb", bufs=2, space="PSUM") as pbp,
        ):
            wpack = cpool.tile([128, 516], f16)
            warm = cpool.tile([128, 1], f32)
            dsrc = cpool.tile([128, NCHUNK], f16)
            biasp = wpack[:, 512:516].bitcast(f32)   # [128, 2] f32
            # weights on the ACT HWDGE ring; x / out on the sync ring so the
            # two streams start in parallel
            nc.scalar.dma_start(out=wpack[:], in_=wpack_d[:, :])
            # load the Square ACT table set during the DMA head, off the
            # critical path (the only ACT func used -> no table reloads)
            nc.scalar.activation(out=warm[:], in_=biasp[:, 0:1], func=Square,
                                 bias=0.0, scale=1.0)
            wA = wpack[:, 0:128]
            wB = wpack[:, 128:256]
            lamA = wpack[:, 256:384]
            lamB = wpack[:, 384:512]
            # dummy matmuls on memset data: lift the HAM clock gate to 8/8
            # while the weight/x DMAs are still in flight, so the real
            # stream runs at 2.4 GHz from its first matmul
            nc.gpsimd.memset(dsrc[:], 0.25)
            for i in range(3):
                pool, tag = (pap, "yA") if i % 2 == 0 else (pbp, "yB")
                dw = pool.tile([128, 2 * NCHUNK], f32, tag=tag)
                for u in range(2):
                    nc.tensor.matmul(
                        out=dw[:, u * NCHUNK:(u + 1) * NCHUNK],
                        lhsT=dsrc[:, 0:128],
                        rhs=dsrc[:],
                        start=True, stop=True,
                    )

            xTs = []
            for T in range(NTILES):
                xT = xtp.tile([128, TILE_F], f16, tag=f"xt{T}",
                              name=f"xt{T}")
                xTs.append(xT)
            # first chunk split out so compute starts after 256 KiB
            nc.sync.dma_start(out=xTs[0][:, 0:1024], in_=xt_d[:, 0:1024])
            nc.sync.dma_start(out=xTs[0][:, 1024:2048],
                              in_=xt_d[:, 1024:2048])
            for T in range(1, NTILES):
                nc.sync.dma_start(
                    out=xTs[T][:], in_=xt_d[:, T * TILE_F:(T + 1) * TILE_F]
                )
            onats = [onp_.tile([128, TILE_F], f16, tag=f"onat{i}",
                               name=f"onat{i}") for i in range(NTILES)]

            DVE_ZB = {0, 4, 7}        # chunks whose B-half squares on DVE

            def m2_block(st, last=False):
                """M2 (into the yA tile, WAR after ACT) + DVE cast +
                per-chunk sync HWDGE DMA out."""
                h, yA, zA, zB = st
                for u in range(2):
                    nc.tensor.matmul(
                        out=yA[:, u * NCHUNK:(u + 1) * NCHUNK],
                        lhsT=lamA,
                        rhs=zA[:, u * NCHUNK:(u + 1) * NCHUNK],
                        start=True, stop=False,
                    )
                for u in range(2):
                    nc.tensor.matmul(
                        out=yA[:, u * NCHUNK:(u + 1) * NCHUNK],
                        lhsT=lamB,
                        rhs=zB[:, u * NCHUNK:(u + 1) * NCHUNK],
                        start=False, stop=True,
                    )
                T, c0 = h // 2, (h % 2) * 1024
                s0 = h * 1024
                if not last:
                    nc.vector.tensor_copy(
                        out=onats[T][:, c0:c0 + 1024], in_=yA[:]
                    )
                    nc.sync.dma_start(
                        out=out[:, s0:s0 + 1024],
                        in_=onats[T][:, c0:c0 + 1024],
                    )
                else:
                    # fine-grained drain of the final chunk
                    for u in range(2):
                        nc.vector.tensor_copy(
                            out=onats[T][:, c0 + u * NCHUNK:
                                         c0 + (u + 1) * NCHUNK],
                            in_=yA[:, u * NCHUNK:(u + 1) * NCHUNK],
                        )
                        nc.sync.dma_start(
                            out=out[:, s0 + u * NCHUNK:
                                    s0 + (u + 1) * NCHUNK],
                            in_=onats[T][:, c0 + u * NCHUNK:
                                         c0 + (u + 1) * NCHUNK],
                        )

            prev = None
            for h in range(NH):           # chunk = 1024 cols = 2 matmuls
                xT = xTs[h // 2]
                c0 = (h % 2) * 1024
                yA = pap.tile([128, 2 * NCHUNK], f32, tag="yA")
                yB = pbp.tile([128, 2 * NCHUNK], f32, tag="yB")
                for u in range(2):
                    nc.tensor.matmul(
                        out=yA[:, u * NCHUNK:(u + 1) * NCHUNK],
                        lhsT=wA,
                        rhs=xT[:, c0 + u * NCHUNK:c0 + (u + 1) * NCHUNK],
                        start=True, stop=True,
                    )
                for u in range(2):
                    nc.tensor.matmul(
                        out=yB[:, u * NCHUNK:(u + 1) * NCHUNK],
                        lhsT=wB,
                        rhs=xT[:, c0 + u * NCHUNK:c0 + (u + 1) * NCHUNK],
                        start=True, stop=True,
                    )
                zA = zp.tile([128, 2 * NCHUNK], f16, tag="zA")
                zB = zp.tile([128, 2 * NCHUNK], f16, tag="zB")
                nc.scalar.activation(out=zA[:], in_=yA[:], func=Square,
                                     bias=biasp[:, 0:1], scale=1.0)
                if h in DVE_ZB:
                    tB = zp.tile([128, 2 * NCHUNK], f16, tag="tB")
                    nc.vector.tensor_scalar(
                        out=tB[:], in0=yB[:], scalar1=biasp[:, 1:2],
                        scalar2=None, op0=Alu.add,
                    )
                    nc.vector.tensor_tensor(
                        out=zB[:], in0=tB[:], in1=tB[:], op=Alu.mult,
                    )
                else:
                    nc.scalar.activation(out=zB[:], in_=yB[:], func=Square,
                                         bias=biasp[:, 1:2], scale=1.0)
                if prev is not None:
                    m2_block(prev)
                prev = (h, yA, zA, zB)
            m2_block(prev, last=True)

    nc.finalize()
    return nc


def _ensure_ntff_hook():
    """Install the axon NTFF profiling hook if the image's antenv lacks it."""
    import sys
    import types
    try:
        from antenv.axon_hooks import get_axon_ntff_profile_hook  # noqa: F401
        return
    except ImportError:
        pass
    try:
        import antenv
        from trn_agent_boot.trn_boot import _ntff_profile_via_ctypes
        mod = types.ModuleType("antenv.axon_hooks")
        store = [None]
        mod.set_axon_ntff_profile_hook = lambda h: store.__setitem__(0, h)
        mod.get_axon_ntff_profile_hook = lambda: store[0]
        sys.modules["antenv.axon_hooks"] = mod
        antenv.axon_hooks = mod
        mod.set_axon_ntff_profile_hook(
            _ntff_profile_via_ctypes("/opt/axon/libaxon_pjrt.so")
        )
        import concourse.bass_utils as bu
        bu.upload_artifacts = lambda tmpdir: tmpdir
    except Exception as e:  # profiling is best-effort
        print(f"ntff hook install failed: {e}")


def kernel(t, state, cmd, coeffs):
    from concourse.bass_utils import run_bass_kernel_spmd

    trace = bool(int(os.environ.get("BOAT_TRACE", "0")))
    if trace:
        _ensure_ntff_hook()

    t = np.asarray(t)
    state16 = np.asarray(state, dtype=np.float16)
    cmd = np.asarray(cmd, dtype=np.float32)
    coeffs = np.asarray(coeffs, dtype=np.float32)

    wA, wB, lamA, lamB, biasp = _host_weights(t, cmd, coeffs)
    wpack = np.concatenate(
        [wA, wB, lamA, lamB, biasp.view(np.float16)], axis=1
    )                                                      # [128, 516] f16

    if "nc" not in _NC_CACHE:
        if int(os.environ.get("BOAT_RAW", "0")):
            _NC_CACHE["nc"] = _build_nc_raw()
        else:
            _NC_CACHE["nc"] = _build_nc()
    nc = _NC_CACHE["nc"]

    in_maps = []
    for k in range(NCORES):
        shard = state16[k * BS:(k + 1) * BS]
        # xt[32bp + 4nh + f, T*2048 + 32bc + j] =
        #    state[(T*128 + 32bp + j)*512 + 8bc + nh, f]
        xt = np.ascontiguousarray(
            shard.reshape(NTILES, 4, 32, 64, 8, 4)      # T bp j bc nh f
            .transpose(1, 4, 5, 0, 3, 2)                # bp nh f T bc j
            .reshape(128, NTILES * TILE_F)
        )
        in_maps.append({"xt": xt, "wpack": wpack})

    res = run_bass_kernel_spmd(
        nc,
        in_maps,
        core_ids=list(range(NCORES)),
        trace=trace,
    )
    LAST_RESULT[0] = res
    outs = []
    for r in res.results:
        # out[(T*128+32bp+j)*512 + 8bc + nh, c] =
        #    o[32bp + 4nh + c, T*2048 + 32bc + j]
        o = (r["out"].reshape(4, 8, 4, NTILES, 64, 32)   # bp nh c T bc j
             .transpose(3, 0, 5, 4, 1, 2)                # T bp j bc nh c
             .reshape(BS, 4))
        outs.append(o)
    return np.concatenate(outs, axis=0).astype(np.float32)



# revision 41
# speedup vs baseline: 1.0375x; 1.0375x over previous
"""Trainium2 Bass kernel for the Boat Dynamic System — rank-8 CP, fp16, v7.

Math: out[b, c] = s~^T Q_c s~ with s~ = (1, u, v, r, Pf); pro/rud folded on
host. The 4x5x5 tensor {Q_c} is CP-decomposed (Gauss-Newton, exact fit) as
Q_c = sum_{j=0..7} lam[c,j] w_j w_j^T, then W is rounded to fp16 and
(lam, bias) are REFIT in f64 so quantization cancels to first order.

v11 balances the elementwise work across ACT and DVE (the only two engines
with a PSUM read port; DVE has ONE such port so tensor ops may read at most
one PSUM operand), moves every DMA to HWDGE (sync/scalar queues), and warms
the PE's HAM clock gate with dummy matmuls on memset data during the DMA
head so the real matmul stream runs at 2.4 GHz instead of 1.2:
  - ACT: Square(y + b) for 13 of the 16 square-chunks (its only func ->
    the activation table never reloads, 1.3us each)
  - DVE: for chunks in DVE_ZB, the B-half square runs two-stage:
    t = fp16(yB + bB) via tensor_scalar (PSUM->SBUF), then zB = t*t via
    tensor_tensor (all-SBUF fp16 -> 2x DVE mode)
  - DVE: all M2-output PSUM->SBUF fp16 casts (tensor_copy)

Per [128, 1024] chunk (8 per core): 2 fp16 matmuls wA -> yA psum, 2 wB ->
yB, ACT/DVE squares -> fp16 SBUF, then (pipelined one chunk behind) 4
accumulating lam matmuls back into yA's banks, DVE cast, sync HWDGE DMA
out per chunk (the last chunk drains at 512 granularity). Host does all
layout permutes + f32 upcast. Weights+biases ride one packed [128,516]
fp16 DMA on the ACT ring (bias columns bitcast to f32 on device).

A raw-bass (no Tile) variant is kept behind BOAT_RAW=1; the Tile scheduler
measured faster (its dynamic ordering absorbs the ACT/DVE latency chain
better than the 8-PSUM-bank static schedule allows).
"""

import os

import numpy as np

NCORES = 8
B = 2097152
BS = B // NCORES          # 262144 rows per core
DT = 0.01
NTILES = 4                # [128, 2048] tiles per core
TILE_F = 2048             # free dim per tile (512 batch x 4 comps)
NCHUNK = 512              # matmul free size (one PSUM f32 bank)
CPT = TILE_F // NCHUNK    # 4 chunks per tile
NFUNC = 8

_NC_CACHE = {}
LAST_RESULT = [None]

# ---------------------------------------------------------------- host math

_MONO_QUAD = [(0, 0), (0, 1), (0, 2), (0, 3), (1, 1), (1, 2), (1, 3),
              (2, 2), (2, 3), (3, 3)]


def _build_Q(t, cmd, coeffs):
    idx = int(np.round(float(np.asarray(t).reshape(-1)[0]) / DT))
    pro = float(cmd[idx, 0])
    rud = float(cmd[idx, 1])
    cf = np.asarray(coeffs, dtype=np.float64)
    ceff = cf[:, 0:15] + pro * cf[:, 15:30] + rud * cf[:, 30:45]  # [4,15]

    Q = np.zeros((4, 5, 5))
    Q[:, 0, 0] = ceff[:, 0]
    for f in range(4):
        Q[:, 0, 1 + f] += ceff[:, 1 + f] / 2
        Q[:, 1 + f, 0] += ceff[:, 1 + f] / 2
    for k, (x, y) in enumerate(_MONO_QUAD):
        m = 5 + k
        if x == y:
            Q[:, 1 + x, 1 + x] += ceff[:, m]
        else:
            Q[:, 1 + x, 1 + y] += ceff[:, m] / 2
            Q[:, 1 + y, 1 + x] += ceff[:, m] / 2
    return Q


def _cp_decompose(Q, N=NFUNC, restarts=40, iters=200, seed=0, tol=1e-11):
    """Gauss-Newton (LM) exact symmetric CP fit: Q_c = sum_j lam_cj w_j w_j^T."""
    rng = np.random.default_rng(seed)
    qn = np.linalg.norm(Q)
    best = None
    iu = np.triu_indices(5)
    wts = np.where(iu[0] == iu[1], 1.0, np.sqrt(2.0))
    a_idx, b_idx = iu

    def resid(W, lam):
        outer = np.einsum('ja,jb->jab', W, W)
        R = Q - np.einsum('cj,jab->cab', lam, outer)
        return (R[:, a_idx, b_idx] * wts).ravel()

    NJ = N * 5 + 4 * N
    for trial in range(restarts):
        W = rng.standard_normal((N, 5))
        lam = rng.standard_normal((4, N)) * 0.1
        mu = 1e-6
        for _ in range(iters):
            r = resid(W, lam)
            res = np.linalg.norm(r)
            J = np.zeros((60, NJ))
            for j in range(N):
                for e in range(5):
                    contrib = (np.where(a_idx == e, W[j, b_idx], 0.0)
                               + np.where(b_idx == e, W[j, a_idx], 0.0)) * wts
                    J[:, j * 5 + e] = (-lam[:, j][:, None]
                                       * contrib[None, :]).ravel()
            outerj = W[:, a_idx] * W[:, b_idx] * wts
            for c in range(4):
                for j in range(N):
                    col = np.zeros((4, 15))
                    col[c] = -outerj[j]
                    J[:, N * 5 + c * N + j] = col.ravel()
            JTJ = J.T @ J
            g = J.T @ r
            ok = False
            for _ in range(40):
                try:
                    step = np.linalg.solve(JTJ + mu * np.eye(NJ), -g)
                except np.linalg.LinAlgError:
                    mu *= 10
                    continue
                Wn = W + step[:N * 5].reshape(N, 5)
                lamn = lam + step[N * 5:].reshape(4, N)
                if np.linalg.norm(resid(Wn, lamn)) < res:
                    W, lam = Wn, lamn
                    mu = max(mu / 3, 1e-12)
                    ok = True
                    break
                mu *= 10
                if mu > 1e12:
                    break
            if not ok:
                break
            if res < tol * qn * 0.1:
                break
        res = np.linalg.norm(resid(W, lam)) / qn
        if res < tol:
            amp = float(np.abs(lam).sum(axis=1).max() * (np.abs(W).max() ** 2))
            if best is None or amp < best[3]:
                best = (W.copy(), lam.copy(), res, amp)
            if trial >= 2 and best[3] < 60:
                break
    if best is None:
        raise RuntimeError("CP decomposition failed to converge")
    return best


def _refit_fp16(Q, W, lam):
    """Round W[:,1:5] to fp16; refit (lam, bias) in f64; quantize lam to
    fp16 with a final bias refit."""
    N = W.shape[0]
    iu = np.triu_indices(5)
    wts = np.where(iu[0] == iu[1], 1.0, np.sqrt(2.0))
    Tq = Q[:, iu[0], iu[1]] * wts                     # [4,15]

    Wf = W.astype(np.float64).copy()
    Wf[:, 1:5] = Wf[:, 1:5].astype(np.float16)
    b = Wf[:, 0].copy()

    def gram(Wfull):
        return np.einsum('ja,jb->jab', Wfull, Wfull)[:, iu[0], iu[1]] * wts

    def fit_lam(Wfull):
        G = gram(Wfull)
        lam2, *_ = np.linalg.lstsq(G.T, Tq.T, rcond=None)
        return lam2.T, G

    def refit_bias(lamx, b):
        for _ in range(100):
            Wf[:, 0] = b
            G = gram(Wf)
            R = Tq - lamx @ G
            J = np.zeros((60, N))
            for j in range(N):
                dG = np.zeros((5, 5))
                dG[0, :] += Wf[j]
                dG[:, 0] += Wf[j]
                J[:, j] = np.outer(lamx[:, j], dG[iu[0], iu[1]] * wts).ravel()
            step, *_ = np.linalg.lstsq(J, R.ravel(), rcond=None)
            b = b + step
            if np.linalg.norm(step) < 1e-13:
                break
        return b

    for _ in range(3):
        Wf[:, 0] = b
        lam2, _ = fit_lam(Wf)
        b = refit_bias(lam2, b)
    lam16 = lam2.astype(np.float16).astype(np.float64)
    b = refit_bias(lam16, b)
    Wf[:, 0] = b
    return Wf, b, lam16


def _host_weights(t, cmd, coeffs):
    """Device weights: 4x [128,128] block-diag kron mats (fp16) + biases.

    biasp columns: 0 = bA (A-half Square bias), 1 = bB (B-half bias).
    """
    Q = _build_Q(t, cmd, coeffs)
    W, lam, res, amp = _cp_decompose(Q)
    Wf, bias, lam16 = _refit_fp16(Q, W, lam)

    I32 = np.eye(32)
    wA = np.kron(I32, Wf[0:4, 1:5].T).astype(np.float16)    # [128,128]
    wB = np.kron(I32, Wf[4:8, 1:5].T).astype(np.float16)
    lamA = np.kron(I32, lam16[:, 0:4].T).astype(np.float16)
    lamB = np.kron(I32, lam16[:, 4:8].T).astype(np.float16)
    biasp = np.stack([
        np.tile(bias[0:4], 32),
        np.tile(bias[4:8], 32),
    ], axis=1).astype(np.float32)                           # [128, 2]
    return wA, wB, lamA, lamB, biasp


# ---------------------------------------------------------------- device

def _build_nc_raw():
    """Raw-bass (no Tile) build: manual semaphores, ~170 instructions.

    Engine programs:
      SP:  3 in-DMAs, then per chunk wait(cast) -> out-DMA, final drain wait
      PE:  per chunk h: M1 (4 mm) -> inc sP1; M2 of h-1 (4 mm) -> inc sP2
      ACT: packed weight DMA, warm Square, per chunk zA (+zB for non-DVE
           chunks), inc sZA / sZB_ACT
      DVE: two-stage zB for DVE_ZB chunks (inc sZB_DVE), cast of chunk h-1
           after sP2 (inc sCast)
    """
    import concourse.bacc as bacc
    import concourse.mybir as mybir

    nc = bacc.Bacc("TRN2", target_bir_lowering=False, debug=False)
    f32 = mybir.dt.float32
    f16 = mybir.dt.float16
    Square = mybir.ActivationFunctionType.Square
    Alu = mybir.AluOpType

    xt_d = nc.dram_tensor("xt", [128, NTILES * TILE_F], f16,
                          kind="ExternalInput")
    # wpack cols 0:512 = wA|wB|lamA|lamB f16; cols 512:516 = biasp bitcast
    # ([128, 2] f32: bA, bB)
    wpack_d = nc.dram_tensor("wpack", [128, 516], f16, kind="ExternalInput")
    out = nc.dram_tensor("out", [128, NTILES * TILE_F], f16,
                         kind="ExternalOutput")

    NH = NTILES * TILE_F // 1024          # 8 chunks of 1024 cols
    DVE_ZB = (0, 7)                       # chunks whose B-half squares on DVE
    NWARM = 8                             # dummy MMs to lift the HAM clock gate

    # zB(h) completion bookkeeping: which engine + that engine's count
    zb_eng = ["dve" if h in DVE_ZB else "act" for h in range(NH)]
    zb_cnt = []
    na = nd = 0
    for h in range(NH):
        if zb_eng[h] == "act":
            na += 1
            zb_cnt.append(na)
        else:
            nd += 1
            zb_cnt.append(nd)

    NYA = 2                               # yA psum ring depth (4 banks)
    NYB = 2                               # yB psum ring depth (4 banks)

    from contextlib import ExitStack
    with ExitStack() as stack:
        ec = stack.enter_context
        sIN0 = ec(nc.semaphore("sIN0"))
        sIN1 = ec(nc.semaphore("sIN1"))
        sIN2 = ec(nc.semaphore("sIN2"))
        sW = ec(nc.semaphore("sW"))
        sOUT = ec(nc.semaphore("sOUT"))
        sP1a = ec(nc.semaphore("sP1a"))   # yA M1 pair done
        sP1b = ec(nc.semaphore("sP1b"))   # yB M1 pair done
        sP2 = ec(nc.semaphore("sP2"))
        sZA = ec(nc.semaphore("sZA"))
        sZBa = ec(nc.semaphore("sZBa"))
        sZBd = ec(nc.semaphore("sZBd"))
        sTB = ec(nc.semaphore("sTB"))
        sCast = ec(nc.semaphore("sCast"))
        xt = ec(nc.sbuf_tensor("xt_s", [128, NTILES * TILE_F], f16))
        wpack = ec(nc.sbuf_tensor("wpack_s", [128, 516], f16))
        onat = ec(nc.sbuf_tensor("onat_s", [128, NTILES * TILE_F], f16))
        zA_s = ec(nc.sbuf_tensor("zA_s", [128, 2 * 1024], f16))
        zB_s = ec(nc.sbuf_tensor("zB_s", [128, 2 * 1024], f16))
        tB_s = ec(nc.sbuf_tensor("tB_s", [128, 2 * 1024], f16))
        warm_s = ec(nc.sbuf_tensor("warm_s", [128, 1], f32))
        yA0 = ec(nc.psum_tensor("yA0", [128, 1024], f32))
        yA1 = ec(nc.psum_tensor("yA1", [128, 1024], f32))
        yB0 = ec(nc.psum_tensor("yB0", [128, 1024], f32))
        yB1 = ec(nc.psum_tensor("yB1", [128, 1024], f32))
        wA = wpack[:, 0:128]
        wB = wpack[:, 128:256]
        lamA = wpack[:, 256:384]
        lamB = wpack[:, 384:512]
        biasp = wpack[:, 512:516].bitcast(f32)   # [128, 2] f32
        yAs = [yA0, yA1]
        yBs = [yB0, yB1]
        zAs = [zA_s[:, 0:1024], zA_s[:, 1024:2048]]
        zBs = [zB_s[:, 0:1024], zB_s[:, 1024:2048]]
        tBs = [tB_s[:, 0:1024], tB_s[:, 1024:2048]]
        # in-DMA completion semaphore per chunk (one sem per DMA: concurrent
        # DMA completions on one sem are unordered)
        in_need = [sIN0, sIN1, sIN1, sIN1, sIN2, sIN2, sIN2, sIN2]

        # ---- SP program: x in, then per-chunk out (weights ride the ACT
        # HWDGE ring in parallel)
        nc.sync.dma_start(out=xt[:, 0:1024],
                          in_=xt_d[:, 0:1024]).then_inc(sIN0, 16)
        nc.sync.dma_start(out=xt[:, 1024:4096],
                          in_=xt_d[:, 1024:4096]).then_inc(sIN1, 16)
        nc.sync.dma_start(out=xt[:, 4096:8192],
                          in_=xt_d[:, 4096:8192]).then_inc(sIN2, 16)
        for h in range(NH - 1):
            nc.sync.wait_ge(sCast, h + 1)
            nc.sync.dma_start(
                out=out[:, h * 1024:(h + 1) * 1024],
                in_=onat[:, h * 1024:(h + 1) * 1024],
            ).then_inc(sOUT, 16)
        # last chunk drains at 512 granularity (casts split likewise)
        for u in range(2):
            nc.sync.wait_ge(sCast, NH + u)
            s0 = (NH - 1) * 1024 + u * 512
            nc.sync.dma_start(
                out=out[:, s0:s0 + 512], in_=onat[:, s0:s0 + 512],
            ).then_inc(sOUT, 16)
        nc.sync.wait_ge(sOUT, 16 * (NH + 1))

        # ---- ACT program
        nc.scalar.dma_start(out=wpack[:], in_=wpack_d[:, :]).then_inc(sW, 16)
        nc.scalar.wait_ge(sW, 16)
        nc.scalar.activation(out=warm_s[:], in_=biasp[:, 0:1], func=Square,
                             bias=0.0, scale=1.0)
        for h in range(NH):
            nc.scalar.wait_ge(sP1a, h + 1)
            if h >= 2:
                nc.scalar.wait_ge(sP2, h - 1)   # zA buf WAR vs M2A(h-2)
            nc.scalar.activation(out=zAs[h % 2], in_=yAs[h % NYA][:],
                                 func=Square, bias=biasp[:, 0:1],
                                 scale=1.0).then_inc(sZA, 1)
            if zb_eng[h] == "act":
                nc.scalar.wait_ge(sP1b, h + 1)
                nc.scalar.activation(out=zBs[h % 2], in_=yBs[h % NYB][:],
                                     func=Square, bias=biasp[:, 1:2],
                                     scale=1.0).then_inc(sZBa, 1)

        # ---- DVE program (casts lead; DVE zB only on fill/tail chunks)
        for h in range(NH):
            if h >= 1:
                nc.vector.wait_ge(sP2, h)
                if h - 1 == NH - 1:
                    break
                nc.vector.tensor_copy(
                    out=onat[:, (h - 1) * 1024:h * 1024],
                    in_=yAs[(h - 1) % NYA][:],
                ).then_inc(sCast, 1)
            if zb_eng[h] == "dve":
                nc.vector.wait_ge(sP1b, h + 1)
                if h >= 2:
                    nc.vector.wait_ge(sP2, h - 1)   # zB buf WAR
                nc.vector.tensor_scalar(
                    out=tBs[h % 2], in0=yBs[h % NYB][:],
                    scalar1=biasp[:, 1:2], scalar2=None, op0=Alu.add,
                ).then_inc(sTB, 1)
                nc.vector.wait_ge(sTB, zb_cnt[h])   # same-engine RAW on tB
                nc.vector.tensor_tensor(
                    out=zBs[h % 2], in0=tBs[h % 2], in1=tBs[h % 2],
                    op=Alu.mult,
                ).then_inc(sZBd, 1)
        nc.vector.wait_ge(sP2, NH)
        for u in range(2):                  # split final cast for the drain
            s0 = (NH - 1) * 1024 + u * 512
            nc.vector.tensor_copy(
                out=onat[:, s0:s0 + 512],
                in_=yAs[(NH - 1) % NYA][:, u * 512:(u + 1) * 512],
            ).then_inc(sCast, 1)

        # ---- PE program
        # dummy matmuls at program start: keep the PE busy from ~7us so the
        # HAM clock gate reaches 8/8 before the real stream begins
        nc.tensor.wait_ge(sW, 16)
        for i in range(NWARM):
            nc.tensor.matmul(
                out=yB0[:, 0:512], lhsT=wA, rhs=wpack[:, 0:512],
                start=True, stop=True,
            )

        def m1(h):
            nc.tensor.wait_ge(in_need[h], 16)
            if h >= NYA:
                nc.tensor.wait_ge(sCast, h - NYA + 1)    # yA(h-NYA) free
            if h >= NYB:                                 # yB(h-NYB) free
                if zb_eng[h - NYB] == "act":
                    nc.tensor.wait_ge(sZBa, zb_cnt[h - NYB])
                else:
                    nc.tensor.wait_ge(sZBd, zb_cnt[h - NYB])
            c0 = h * 1024
            for u in range(2):
                mm = nc.tensor.matmul(
                    out=yAs[h % NYA][:, u * 512:(u + 1) * 512], lhsT=wA,
                    rhs=xt[:, c0 + u * 512:c0 + (u + 1) * 512],
                    start=True, stop=True,
                )
            mm.then_inc(sP1a, 1)
            for u in range(2):
                mm = nc.tensor.matmul(
                    out=yBs[h % NYB][:, u * 512:(u + 1) * 512], lhsT=wB,
                    rhs=xt[:, c0 + u * 512:c0 + (u + 1) * 512],
                    start=True, stop=True,
                )
            mm.then_inc(sP1b, 1)

        def m2(h):
            nc.tensor.wait_ge(sZA, h + 1)
            if zb_eng[h] == "act":
                nc.tensor.wait_ge(sZBa, zb_cnt[h])
            else:
                nc.tensor.wait_ge(sZBd, zb_cnt[h])
            for u in range(2):
                nc.tensor.matmul(
                    out=yAs[h % NYA][:, u * 512:(u + 1) * 512], lhsT=lamA,
                    rhs=zAs[h % 2][:, u * 512:(u + 1) * 512],
                    start=True, stop=False,
                )
            for u in range(2):
                mm = nc.tensor.matmul(
                    out=yAs[h % NYA][:, u * 512:(u + 1) * 512], lhsT=lamB,
                    rhs=zBs[h % 2][:, u * 512:(u + 1) * 512],
                    start=False, stop=True,
                )
            mm.then_inc(sP2, 1)

        for h in range(NH):
            m1(h)
            if h >= 1:
                m2(h - 1)
        m2(NH - 1)

    nc.finalize()
    return nc


def _build_nc():
    import concourse.bacc as bacc
    import concourse.mybir as mybir
    import concourse.tile as tile

    nc = bacc.Bacc("TRN2", target_bir_lowering=False, debug=False)
    f32 = mybir.dt.float32
    f16 = mybir.dt.float16
    Square = mybir.ActivationFunctionType.Square
    Alu = mybir.AluOpType

    # [q, (T g)]: per tile, 4KB contiguous per partition
    xt_d = nc.dram_tensor("xt", [128, NTILES * TILE_F], f16,
                          kind="ExternalInput")
    wpack_d = nc.dram_tensor("wpack", [128, 516], f16, kind="ExternalInput")
    out = nc.dram_tensor("out", [128, NTILES * TILE_F], f16,
                         kind="ExternalOutput")

    NH = NTILES * TILE_F // 1024          # 8 chunks of 1024 cols

    with tile.TileContext(nc) as tc:
        with (
            tc.tile_pool(name="consts", bufs=1) as cpool,
            tc.tile_pool(name="xt", bufs=4) as xtp,
            tc.tile_pool(name="z", bufs=2) as zp,
            tc.tile_pool(name="onat", bufs=4) as onp_,
            tc.tile_pool(name="pa", bufs=2, space="PSUM") as pap,
            tc.tile_pool(name="pb", bufs=2, space="PSUM") as pbp,
        ):
            wpack = cpool.tile([128, 516], f16)
            warm = cpool.tile([128, 1], f32)
            dsrc = cpool.tile([128, NCHUNK], f16)
            biasp = wpack[:, 512:516].bitcast(f32)   # [128, 2] f32
            # weights on the ACT HWDGE ring; x / out on the sync ring so the
            # two streams start in parallel
            nc.scalar.dma_start(out=wpack[:], in_=wpack_d[:, :])
            # load the Square ACT table set during the DMA head, off the
            # critical path (the only ACT func used -> no table reloads)
            nc.scalar.activation(out=warm[:], in_=biasp[:, 0:1], func=Square,
                                 bias=0.0, scale=1.0)
            wA = wpack[:, 0:128]
            wB = wpack[:, 128:256]
            lamA = wpack[:, 256:384]
            lamB = wpack[:, 384:512]
            # dummy matmuls on memset data: lift the HAM clock gate to 8/8
            # while the weight/x DMAs are still in flight, so the real
            # stream runs at 2.4 GHz from its first matmul
            nc.gpsimd.memset(dsrc[:], 0.25)
            for i in range(3):
                pool, tag = (pap, "yA") if i % 2 == 0 else (pbp, "yB")
                dw = pool.tile([128, 2 * NCHUNK], f32, tag=tag)
                for u in range(2):
                    nc.tensor.matmul(
                        out=dw[:, u * NCHUNK:(u + 1) * NCHUNK],
                        lhsT=dsrc[:, 0:128],
                        rhs=dsrc[:],
                        start=True, stop=True,
                    )

            xTs = []
            for T in range(NTILES):
                xT = xtp.tile([128, TILE_F], f16, tag=f"xt{T}",
                              name=f"xt{T}")
                xTs.append(xT)
            # first chunk split out so compute starts after 256 KiB
            nc.sync.dma_start(out=xTs[0][:, 0:1024], in_=xt_d[:, 0:1024])
            nc.sync.dma_start(out=xTs[0][:, 1024:2048],
                              in_=xt_d[:, 1024:2048])
            for T in range(1, NTILES):
                nc.sync.dma_start(
                    out=xTs[T][:], in_=xt_d[:, T * TILE_F:(T + 1) * TILE_F]
                )
            onats = [onp_.tile([128, TILE_F], f16, tag=f"onat{i}",
                               name=f"onat{i}") for i in range(NTILES)]

            DVE_ZB = {0, 4, 7}        # chunks whose B-half squares on DVE

            def m2_block(st, last=False):
                """M2 (into the yA tile, WAR after ACT) + DVE cast +
                per-chunk sync HWDGE DMA out."""
                h, yA, zA, zB = st
                for u in range(2):
                    nc.tensor.matmul(
                        out=yA[:, u * NCHUNK:(u + 1) * NCHUNK],
                        lhsT=lamA,
                        rhs=zA[:, u * NCHUNK:(u + 1) * NCHUNK],
                        start=True, stop=False,
                    )
                for u in range(2):
                    nc.tensor.matmul(
                        out=yA[:, u * NCHUNK:(u + 1) * NCHUNK],
                        lhsT=lamB,
                        rhs=zB[:, u * NCHUNK:(u + 1) * NCHUNK],
                        start=False, stop=True,
                    )
                T, c0 = h // 2, (h % 2) * 1024
                s0 = h * 1024
                # 512-wide casts: each half depends only on its own M2
                # accumulation group, freeing yA's banks sooner (the cast
                # sits in the loop-carried PSUM-reuse cycle)
                for u in range(2):
                    nc.vector.tensor_copy(
                        out=onats[T][:, c0 + u * NCHUNK:
                                     c0 + (u + 1) * NCHUNK],
                        in_=yA[:, u * NCHUNK:(u + 1) * NCHUNK],
                    )
                    if last:
                        nc.sync.dma_start(
                            out=out[:, s0 + u * NCHUNK:
                                    s0 + (u + 1) * NCHUNK],
                            in_=onats[T][:, c0 + u * NCHUNK:
                                         c0 + (u + 1) * NCHUNK],
                        )
                if not last:
                    nc.sync.dma_start(
                        out=out[:, s0:s0 + 1024],
                        in_=onats[T][:, c0:c0 + 1024],
                    )

            prev = None
            for h in range(NH):           # chunk = 1024 cols = 2 matmuls
                xT = xTs[h // 2]
                c0 = (h % 2) * 1024
                yA = pap.tile([128, 2 * NCHUNK], f32, tag="yA")
                yB = pbp.tile([128, 2 * NCHUNK], f32, tag="yB")
                for u in range(2):
                    nc.tensor.matmul(
                        out=yA[:, u * NCHUNK:(u + 1) * NCHUNK],
                        lhsT=wA,
                        rhs=xT[:, c0 + u * NCHUNK:c0 + (u + 1) * NCHUNK],
                        start=True, stop=True,
                    )
                for u in range(2):
                    nc.tensor.matmul(
                        out=yB[:, u * NCHUNK:(u + 1) * NCHUNK],
                        lhsT=wB,
                        rhs=xT[:, c0 + u * NCHUNK:c0 + (u + 1) * NCHUNK],
                        start=True, stop=True,
                    )
                zA = zp.tile([128, 2 * NCHUNK], f16, tag="zA")
                zB = zp.tile([128, 2 * NCHUNK], f16, tag="zB")
                nc.scalar.activation(out=zA[:], in_=yA[:], func=Square,
                                     bias=biasp[:, 0:1], scale=1.0)
                if h in DVE_ZB:
                    tB = zp.tile([128, 2 * NCHUNK], f16, tag="tB")
                    nc.vector.tensor_scalar(
                        out=tB[:], in0=yB[:], scalar1=biasp[:, 1:2],
                        scalar2=None, op0=Alu.add,
                    )
                    nc.vector.tensor_tensor(
                        out=zB[:], in0=tB[:], in1=tB[:], op=Alu.mult,
                    )
                else:
                    nc.scalar.activation(out=zB[:], in_=yB[:], func=Square,
                                         bias=biasp[:, 1:2], scale=1.0)
                if prev is not None:
                    m2_block(prev)
                prev = (h, yA, zA, zB)
            m2_block(prev, last=True)

    nc.finalize()
    return nc


def _ensure_ntff_hook():
    """Install the axon NTFF profiling hook if the image's antenv lacks it."""
    import sys
    import types
    try:
        from antenv.axon_hooks import get_axon_ntff_profile_hook  # noqa: F401
        return
    except ImportError:
        pass
    try:
        import antenv
        from trn_agent_boot.trn_boot import _ntff_profile_via_ctypes
        mod = types.ModuleType("antenv.axon_hooks")
        store = [None]
        mod.set_axon_ntff_profile_hook = lambda h: store.__setitem__(0, h)
        mod.get_axon_ntff_profile_hook = lambda: store[0]
        sys.modules["antenv.axon_hooks"] = mod
        antenv.axon_hooks = mod
        mod.set_axon_ntff_profile_hook(
            _ntff_profile_via_ctypes("/opt/axon/libaxon_pjrt.so")
        )
        import concourse.bass_utils as bu
        bu.upload_artifacts = lambda tmpdir: tmpdir
    except Exception as e:  # profiling is best-effort
        print(f"ntff hook install failed: {e}")


def kernel(t, state, cmd, coeffs):
    from concourse.bass_utils import run_bass_kernel_spmd

    trace = bool(int(os.environ.get("BOAT_TRACE", "0")))
    if trace:
        _ensure_ntff_hook()

    t = np.asarray(t)
    state16 = np.asarray(state, dtype=np.float16)
    cmd = np.asarray(cmd, dtype=np.float32)
    coeffs = np.asarray(coeffs, dtype=np.float32)

    wA, wB, lamA, lamB, biasp = _host_weights(t, cmd, coeffs)
    wpack = np.concatenate(
        [wA, wB, lamA, lamB, biasp.view(np.float16)], axis=1
    )                                                      # [128, 516] f16

    if "nc" not in _NC_CACHE:
        if int(os.environ.get("BOAT_RAW", "0")):
            _NC_CACHE["nc"] = _build_nc_raw()
        else:
            _NC_CACHE["nc"] = _build_nc()
    nc = _NC_CACHE["nc"]

    in_maps = []
    for k in range(NCORES):
        shard = state16[k * BS:(k + 1) * BS]
        # xt[32bp + 4nh + f, T*2048 + 32bc + j] =
        #    state[(T*128 + 32bp + j)*512 + 8bc + nh, f]
        xt = np.ascontiguousarray(
            shard.reshape(NTILES, 4, 32, 64, 8, 4)      # T bp j bc nh f
            .transpose(1, 4, 5, 0, 3, 2)                # bp nh f T bc j
            .reshape(128, NTILES * TILE_F)
        )
        in_maps.append({"xt": xt, "wpack": wpack})

    res = run_bass_kernel_spmd(
        nc,
        in_maps,
        core_ids=list(range(NCORES)),
        trace=trace,
    )
    LAST_RESULT[0] = res
    outs = []
    for r in res.results:
        # out[(T*128+32bp+j)*512 + 8bc + nh, c] =
        #    o[32bp + 4nh + c, T*2048 + 32bc + j]
        o = (r["out"].reshape(4, 8, 4, NTILES, 64, 32)   # bp nh c T bc j
             .transpose(3, 0, 5, 4, 1, 2)                # T bp j bc nh c
             .reshape(BS, 4))
        outs.append(o)
    return np.concatenate(outs, axis=0).astype(np.float32)

